# revision 11
# baseline (speedup 1.0000x reference)
"""Trainium2 Bass kernel for a 4-layer transformer decoder (self-attn +
cross-attn + FFN, post-residual, exact GELU), distributed over 8 NeuronCores.

Sharding: data-parallel over batch (B=4 -> 4 core pairs); within a pair the
target sequence T=1024 is split in half (512 rows per core). Activations are
kept feature-major ("transposed", [D, T_half]) so every projection is a
single matmul chain with no transposes. Per layer each core computes its own
self-attn K/V shard (its 512 target rows) AND its cross-attn K/V shard (its
512 encoder rows), exchanged with its pair via two AllGathers (bf16) that
overlap projection compute. Both attentions then consume identical
kT-wave/v-augmented layouts from the gathered DRAM buffers.

Projections run in fp32r (full PE rate at N=512); attention score/AV matmuls
and the K/V projections run in bf16 with fp32 PSUM accumulation. Score
matmul pairs (K=64) sit on distinct PE row-strips (base partitions 0/64) and
execute concurrently. The attention inner loop is software-pipelined
(scores[sc+1] issued before AV[sc]) to hide softmax-exp latency; row-sum
reciprocals use the fast approximate DVE op. Softmax skips max-subtraction
(scores are O(1)) and row sums come free from an appended ones-column on V.
The fp32 residual stream stays in SBUF for all 4 layers. All biases in the
reference are zero and are folded out.

Self-contained: hardcodes all shapes; no file I/O, no sibling imports.
"""
import numpy as np
import ml_dtypes

import concourse.bass as bass
import concourse.mybir as mybir
import concourse.tile as tile
from concourse import bacc
from concourse import bass_utils

F32 = mybir.dt.float32
F32R = mybir.dt.float32r
BF16 = mybir.dt.bfloat16
EXP = mybir.ActivationFunctionType.Exp
GELU = mybir.ActivationFunctionType.Gelu

L, D, H, DK, HID = 4, 1024, 16, 64, 4096
B, T, S = 4, 1024, 1024
R = T // 2              # rows (target positions / encoder positions) per core
N_CORES = 8
DC = D // 128           # 8 feature chunks
SC = S // 128           # 8 key chunks
RC = R // 128           # 4 own-row chunks
KV_ELEMS = D * R + R * D        # kT flat + v flat per-core shard (bf16)
RG = [[0, 1], [2, 3], [4, 5], [6, 7]]

_CACHE = {}


def _emit(nc, tc, pools, dram):
    (xp, xbp, ep, wp, wkp, wbp, qp, kvp, kwp, vap, avp, hp, accp,
     minip, minir, minib) = pools

    def dma(dst, src):
        nc.sync.dma_start(dst, src)

    def dma_s(dst, src):
        # store queue on the Activation HWDGE: keeps collective-input stores
        # out of the weight-prefetch queue (ACT is idle during the KV phases)
        nc.scalar.dma_start(dst, src)

    def proj_T(w_ap, rhs_tiles, oc_list, col0, consume, ppool, og=4):
        """Transposed-mode projection: psum[oc] = sum_kc
        W[kc*128:+128, col0+oc*128:+128].T @ rhs[kc]; consume(oc, psum)."""
        kcn = len(rhs_tiles)
        bf = rhs_tiles[0].dtype == BF16
        for g0 in range(0, len(oc_list), og):
            grp = oc_list[g0:g0 + og]
            psums = []
            for _ in grp:
                ps = ppool.tile([128, R], F32, tag="pj")
                psums.append(ps)
            for kc in range(kcn):
                if bf:
                    ws = wbp.tile([128, 512], BF16, tag="wsb")
                else:
                    ws = wp.tile([128, 512], F32R, tag="ws")
                c0 = col0 + grp[0] * 128
                dma(ws[:, 0:len(grp) * 128],
                    w_ap[kc * 128:(kc + 1) * 128, c0:c0 + len(grp) * 128])
                for j in range(len(grp)):
                    nc.tensor.matmul(
                        psums[j][:], ws[:, j * 128:(j + 1) * 128],
                        rhs_tiles[kc][:],
                        start=(kc == 0), stop=(kc == kcn - 1))
            for j, oc in enumerate(grp):
                consume(oc, psums[j])

    def attention(q_tiles, kT_of_wave, va_tiles, spool, avpool, escale):
        """Feature-major attention, software-pipelined over (wave, sc).
        Returns DC f32r [128,R] avT tiles."""
        avT = []
        for wi in range(H // 2):
            at = avp.tile([128, R], F32R, tag="avt", name=f"avt{wi}")
            avT.append(at)
        NW = H // 2
        kw = [None] * NW
        avs = [None] * NW
        pend = None  # (w, sc, p_t)

        def flush(p):
            w_, sc_, pt_ = p
            for hi in range(2):
                nc.tensor.matmul(
                    avs[w_][hi][0:65, :],
                    va_tiles[sc_][:, (2 * w_ + hi) * 65:(2 * w_ + hi + 1) * 65],
                    pt_[:, hi * R:(hi + 1) * R],
                    start=(sc_ == 0), stop=(sc_ == SC - 1))
            if sc_ == SC - 1:
                for hi in range(2):
                    rec = minir.tile([1, R], F32, tag="rec")
                    nc.vector.reciprocal(rec[:], avs[w_][hi][64:65, :])
                    bc = minib.tile([64, R], F32, tag="bc")
                    nc.gpsimd.partition_broadcast(bc[:], rec[:])
                    nc.vector.tensor_mul(
                        avT[w_][hi * 64:(hi + 1) * 64, :],
                        avs[w_][hi][0:64, :], bc[:])

        for w in range(NW):
            kw[w] = kT_of_wave(w)
            avs[w] = [avpool.tile([128, R], F32, tag="av", name=f"av{w}_{i}")
                      for i in range(2)]
            for sc in range(SC):
                slab = spool.tile([128, 2 * R], F32, tag="sc")
                p_t = minip.tile([128, 2 * R], BF16, tag="p")
                for hi in range(2):
                    nc.tensor.matmul(
                        slab[:, hi * R:(hi + 1) * R],
                        kw[w][hi * 64:(hi + 1) * 64, sc * 128:(sc + 1) * 128],
                        q_tiles[w][hi * 64:(hi + 1) * 64, :],
                        start=True, stop=True)
                if pend is not None:
                    flush(pend)
                nc.scalar.activation(p_t[:], slab[:], EXP, scale=escale)
                pend = (w, sc, p_t)
        flush(pend)
        return avT

    def phase_kv(li, rhs_tiles, w_ap, k_col0, v_col0, ccin, pname):
        """Project rhs (feature-major tiles) into a K^T [D, 512] shard and a
        V [512, D] shard, stored to ccin for the pair AllGather."""
        with tc.tile_pool(name=pname, bufs=8, space="PSUM") as pA:
            def mk_k(oc, ps):
                kt = kvp.tile([128, R], BF16, tag="ko")
                nc.vector.tensor_copy(kt[:], ps[:])
                dma_s(ccin[oc * 128 * R:(oc + 1) * 128 * R]
                      .rearrange("(p s) -> p s", p=128), kt[:])
            proj_T(w_ap, rhs_tiles, list(range(DC)), k_col0, mk_k, pA)

            for vc in range(2):
                psv = []
                for _ in range(RC):
                    ps = pA.tile([128, 512], F32, tag="pj")
                    psv.append(ps)
                for kc in range(DC):
                    ws = wbp.tile([128, 512], BF16, tag="wsb")
                    dma(ws[:], w_ap[kc * 128:(kc + 1) * 128,
                                    v_col0 + vc * 512: v_col0 + (vc + 1) * 512])
                    for t_ in range(RC):
                        nc.tensor.matmul(
                            psv[t_][:], rhs_tiles[kc][:, t_ * 128:(t_ + 1) * 128],
                            ws[:], start=(kc == 0), stop=(kc == DC - 1))
                for t_ in range(RC):
                    vt = kvp.tile([128, 512], BF16, tag="vo")
                    nc.vector.tensor_copy(vt[:], psv[t_][:])
                    rbase = D * R + t_ * 128 * D
                    dst = (ccin[rbase:rbase + 128 * D]
                           .rearrange("(p f) -> p f", f=D)
                           [:, vc * 512:(vc + 1) * 512])
                    dma_s(dst, vt[:])

    def phase_q(li, xT, w_ap, col0, pname):
        qT = [None] * DC
        with tc.tile_pool(name=pname, bufs=8, space="PSUM") as pQ:
            def mk_q(oc, ps):
                t = qp.tile([128, R], BF16, tag="q")
                nc.vector.tensor_copy(t[:], ps[:])
                qT[oc] = t
            proj_T(w_ap, xT, list(range(DC)), col0, mk_q, pQ)
        return qT

    def phase_attn(li, qT, ccout, escale, pname):
        """Attention over a gathered KV buffer: blk0 = pair-even rows,
        blk1 = pair-odd rows."""
        va = []
        for sc in range(SC):
            sav = vap.tile([128, H * 65], BF16, tag="sav")
            sav3 = sav[:].rearrange("p (h w) -> p h w", w=65)
            nc.gpsimd.memset(sav3[:, :, 64:65], 1.0)
            blk = sc // 4
            rbase = blk * KV_ELEMS + D * R + (sc % 4) * 128 * D
            src = (ccout[rbase:rbase + 128 * D]
                   .rearrange("(p f) -> p f", f=D)
                   .rearrange("p (h w) -> p h w", w=DK))
            dma(sav3[:, :, 0:DK], src)
            va.append(sav)

        def kT_wave(w):
            kw = kwp.tile([128, S], BF16, tag="kw")
            for blk in range(2):
                base = blk * KV_ELEMS + w * 128 * R
                dma(kw[:, blk * R:(blk + 1) * R],
                    ccout[base:base + 128 * R]
                    .rearrange("(p s) -> p s", p=128))
            return kw

        with (
            tc.tile_pool(name=f"ps{pname}{li}", bufs=2, space="PSUM") as sD,
            tc.tile_pool(name=f"pa{pname}{li}", bufs=4, space="PSUM") as aD,
        ):
            return attention(qT, kT_wave, va, sD, aD, escale)

    def phase_proj_res(li, name, w_ap, rhs_tiles, res_tiles, shadow=False):
        """x_out = W.T @ rhs + res; returns new x tiles (+bf16 shadows).

        kc-major emission: all 8 output psums live at once, contraction
        chunk as the outer loop — so the in-order PE queue runs 56 of 64
        matmuls before the one that needs the last (late) rhs tile."""
        xo = [None] * DC
        xob = [None] * DC
        with tc.tile_pool(name=f"ps{name}{li}", bufs=8, space="PSUM") as pp:
            psums = []
            for oc in range(DC):
                ps = pp.tile([128, R], F32, tag="pj", name=f"pk{oc}")
                psums.append(ps)
            for kc in range(DC):
                ws = wkp.tile([128, D], F32R, tag="wsk")
                dma(ws[:], w_ap[kc * 128:(kc + 1) * 128, 0:D])
                for oc in range(DC):
                    nc.tensor.matmul(
                        psums[oc][:], ws[:, oc * 128:(oc + 1) * 128],
                        rhs_tiles[kc][:],
                        start=(kc == 0), stop=(kc == DC - 1))
            for oc in range(DC):
                t = xp.tile([128, R], F32R, tag="x")
                nc.vector.tensor_add(t[:], psums[oc][:],
                                     res_tiles[oc][:].bitcast(F32))
                xo[oc] = t
                if shadow:
                    tb = xbp.tile([128, R], BF16, tag="x2b")
                    nc.vector.tensor_copy(tb[:], t[:].bitcast(F32))
                    xob[oc] = tb
        return (xo, xob) if shadow else xo

    def phase_ffn(li, wf1, wf2, x2, x2b):
        acc = [None] * DC
        with tc.tile_pool(name=f"psI{li}", bufs=8, space="PSUM") as pI:
            for qtr in range(4):
                hq = [None] * DC
                def mk_h(oc, ps, hq=hq):
                    t = hp.tile([128, R], BF16, tag="h")
                    nc.scalar.activation(t[:], ps[:], GELU)
                    hq[oc] = t
                proj_T(wf1, x2b, list(range(DC)), qtr * D, mk_h, pI)
                wf2q = wf2[qtr * D:(qtr + 1) * D, :]
                def mk_acc(oc, ps, qtr=qtr):
                    if qtr == 0:
                        t = accp.tile([128, R], F32, tag="acc")
                        nc.vector.tensor_add(t[:], ps[:],
                                             x2[oc][:].bitcast(F32))
                        acc[oc] = t
                    else:
                        nc.vector.tensor_add(acc[oc][:], ps[:], acc[oc][:])
                proj_T(wf2q, hq, list(range(DC)), 0, mk_acc, pI)
        x3 = [None] * DC
        x3b = [None] * DC
        for oc in range(DC):
            xt3 = xp.tile([128, R], F32R, tag="x")
            nc.vector.tensor_copy(xt3[:], acc[oc][:])
            x3[oc] = xt3
            xb3 = xbp.tile([128, R], BF16, tag="xb")
            nc.vector.tensor_copy(xb3[:], acc[oc][:])
            x3b[oc] = xb3
        return x3, x3b

    # ---------------- main program ----------------
    (xT_d, xTb_d, encTh_d, w_sa_qkv, w_sa_out, w_ca_q, w_ca_kv, w_ca_out,
     w_ff1, w_ff2, out_d, cc_in, cc_out, cc2_in, cc2_out) = dram

    xT = []
    xTb = []
    for ci in range(DC):
        xt = xp.tile([128, R], F32R, tag="x")
        dma(xt[:], xT_d.ap()[ci * 128:(ci + 1) * 128])
        xT.append(xt)
        xtb = xbp.tile([128, R], BF16, tag="xb")
        dma(xtb[:], xTb_d.ap()[ci * 128:(ci + 1) * 128])
        xTb.append(xtb)
    encTh = []
    for ci in range(DC):
        et = ep.tile([128, R], BF16, tag="enc")
        dma(et[:], encTh_d.ap()[ci * 128:(ci + 1) * 128])
        encTh.append(et)

    for li in range(L):
        ccin = cc_in[li].ap()
        ccout = cc_out[li].ap()
        ccin2 = cc2_in[li].ap()
        ccout2 = cc2_out[li].ap()
        phase_kv(li, xTb, w_sa_qkv.ap()[li], D, 2 * D, ccin, f"psA{li}")
        nc.gpsimd.collective_compute(
            "AllGather", mybir.AluOpType.bypass, replica_groups=RG,
            ins=[ccin], outs=[ccout])
        phase_kv(li, encTh, w_ca_kv.ap()[li], 0, D, ccin2, f"psC{li}")
        qT = phase_q(li, xTb, w_sa_qkv.ap()[li], 0, f"psQ{li}")
        avT = phase_attn(li, qT, ccout, 0.125, "D")
        # AG2 is emitted after the SA attention so the collective (which
        # blocks the GpSimd queue until it completes) does not stall the
        # attention's partition-broadcasts. Its input stores finished during
        # the KV phases; its output is first needed at the CA attention.
        nc.gpsimd.collective_compute(
            "AllGather", mybir.AluOpType.bypass, replica_groups=RG,
            ins=[ccin2], outs=[ccout2])
        x1 = phase_proj_res(li, "E", w_sa_out.ap()[li], avT, xT)
        caqT = phase_q(li, x1, w_ca_q.ap()[li], 0, f"psF{li}")
        ca_avT = phase_attn(li, caqT, ccout2, 0.125, "G")
        x2, x2b = phase_proj_res(li, "H", w_ca_out.ap()[li], ca_avT, x1,
                                 shadow=True)
        xT, xTb = phase_ffn(li, w_ff1.ap()[li], w_ff2.ap()[li], x2, x2b)

    for oc in range(DC):
        dma(out_d.ap()[oc * 128:(oc + 1) * 128], xT[oc][:].bitcast(F32))


def _build():
    nc = bacc.Bacc("TRN2", target_bir_lowering=False, debug=False,
                   num_devices=N_CORES)
    dram = (
        nc.dram_tensor("xT", [D, R], F32R, kind="ExternalInput"),
        nc.dram_tensor("xTb", [D, R], BF16, kind="ExternalInput"),
        nc.dram_tensor("encTh", [D, R], BF16, kind="ExternalInput"),
        nc.dram_tensor("w_sa_qkv", [L, D, 3 * D], BF16, kind="ExternalInput"),
        nc.dram_tensor("w_sa_out", [L, D, D], F32R, kind="ExternalInput"),
        nc.dram_tensor("w_ca_q", [L, D, D], F32R, kind="ExternalInput"),
        nc.dram_tensor("w_ca_kv", [L, D, 2 * D], BF16, kind="ExternalInput"),
        nc.dram_tensor("w_ca_out", [L, D, D], F32R, kind="ExternalInput"),
        nc.dram_tensor("w_ff1", [L, D, HID], BF16, kind="ExternalInput"),
        nc.dram_tensor("w_ff2", [L, HID, D], BF16, kind="ExternalInput"),
        nc.dram_tensor("out", [D, R], F32, kind="ExternalOutput"),
        [nc.dram_tensor(f"cc_in{i}", [KV_ELEMS], BF16, kind="Internal")
         for i in range(L)],
        [nc.dram_tensor(f"cc_out{i}", [2 * KV_ELEMS], BF16, kind="Internal")
         for i in range(L)],
        [nc.dram_tensor(f"cc2_in{i}", [KV_ELEMS], BF16, kind="Internal")
         for i in range(L)],
        [nc.dram_tensor(f"cc2_out{i}", [2 * KV_ELEMS], BF16, kind="Internal")
         for i in range(L)],
    )
    with tile.TileContext(nc) as tc:
        with (
            tc.tile_pool(name="xp", bufs=13) as xp,
            tc.tile_pool(name="xbp", bufs=8) as xbp,
            tc.tile_pool(name="ep", bufs=8) as ep,       # bf16 [128,R] encTh
            tc.tile_pool(name="wp", bufs=8) as wp,       # f32r [128,512] w slabs
            tc.tile_pool(name="wkp", bufs=3) as wkp,     # f32r [128,D] w slabs
            tc.tile_pool(name="wbp", bufs=12) as wbp,    # bf16 [128,512] w slabs
            tc.tile_pool(name="qp", bufs=8) as qp,       # bf16 [128,R] qT/caqT
            tc.tile_pool(name="kvp", bufs=3) as kvp,     # bf16 kv staging
            tc.tile_pool(name="kwp", bufs=3) as kwp,     # bf16 [128,S] kT wave
            tc.tile_pool(name="vap", bufs=8) as vap,     # bf16 [128,H*65] v_aug
            tc.tile_pool(name="avp", bufs=8) as avp,     # f32r [128,R] avT
            tc.tile_pool(name="hp", bufs=8) as hp,       # bf16 [128,R] ffn hid
            tc.tile_pool(name="accp", bufs=8) as accp,   # f32 [128,R] ffn acc
            tc.tile_pool(name="minip", bufs=4) as minip,  # bf16 p slabs
            tc.tile_pool(name="minir", bufs=2) as minir,  # rec rows
            tc.tile_pool(name="minib", bufs=2) as minib,  # bcast tiles
        ):
            pools = (xp, xbp, ep, wp, wkp, wbp, qp, kvp, kwp, vap, avp, hp,
                     accp, minip, minir, minib)
            _emit(nc, tc, pools, dram)
    nc.compile()
    return nc


def _get_nc():
    if "nc" not in _CACHE:
        _CACHE["nc"] = _build()
    return _CACHE["nc"]


def _prep_in_maps(inputs):
    tgt = np.asarray(inputs["tgt"], dtype=np.float32)
    enc_out = np.asarray(inputs["enc_out"], dtype=np.float32)
    shared = {
        "w_sa_qkv": np.asarray(inputs["sa_qkv_w"]).astype(ml_dtypes.bfloat16),
        "w_sa_out": np.ascontiguousarray(inputs["sa_out_w"], dtype=np.float32),
        "w_ca_q": np.ascontiguousarray(inputs["ca_q_w"], dtype=np.float32),
        "w_ca_kv": np.asarray(inputs["ca_kv_w"]).astype(ml_dtypes.bfloat16),
        "w_ca_out": np.ascontiguousarray(inputs["ca_out_w"], dtype=np.float32),
        "w_ff1": np.asarray(inputs["ff_w1"]).astype(ml_dtypes.bfloat16),
        "w_ff2": np.asarray(inputs["ff_w2"]).astype(ml_dtypes.bfloat16),
    }
    in_maps = []
    for c in range(N_CORES):
        b, hh = c // 2, c % 2
        m = {
            "xT": np.ascontiguousarray(tgt[b].T[:, hh * R:(hh + 1) * R]),
            "xTb": np.ascontiguousarray(
                tgt[b].T[:, hh * R:(hh + 1) * R]).astype(ml_dtypes.bfloat16),
            "encTh": np.ascontiguousarray(
                enc_out[b].T[:, hh * R:(hh + 1) * R]).astype(ml_dtypes.bfloat16),
        }
        m.update(shared)
        in_maps.append(m)
    return in_maps


def kernel(**inputs):
    nc = _get_nc()
    in_maps = _prep_in_maps(inputs)
    res = bass_utils.run_bass_kernel_spmd(nc, in_maps,
                                          core_ids=list(range(N_CORES)))
    out = np.empty((B, T, D), dtype=np.float32)
    for c in range(N_CORES):
        b, hh = c // 2, c % 2
        out[b, hh * R:(hh + 1) * R, :] = res.results[c]["out"].T
    return out


# revision 13
# speedup vs baseline: 1.1102x; 1.1102x over previous
"""Trainium2 Bass kernel for a 4-layer transformer decoder (self-attn +
cross-attn + FFN, post-residual, exact GELU), distributed over 8 NeuronCores.

Sharding: data-parallel over batch (B=4 -> 4 core pairs); within a pair the
target sequence T=1024 is split in half (512 rows per core). Activations are
kept feature-major ("transposed", [D, T_half]) so every projection is a
single matmul chain with no transposes. Per layer each core computes its own
self-attn K/V shard (its 512 target rows) AND its cross-attn K/V shard (its
512 encoder rows), exchanged with its pair via two AllGathers (bf16). The
second AllGather is emitted after the SA attention so it does not block the
GpSimd queue (partition-broadcasts) during the SA softmax window.

All projections except the FFN run in fp8(e4m3) with the DoubleRow perf mode
(two 128-row contraction chunks per instruction): QKV / cross-KV use weights
pre-scaled by 32 (folded out through the softmax: exp-scale and the
ones-column of V), out-projections use unscaled fp8 weights. The FFN stays
bf16 (fp8 there fails the 2e-2 accuracy budget). Attention score/AV matmuls
run in bf16 with fp32 PSUM; score pairs (K=64) sit on distinct PE row-strips
(base partitions 0/64) and execute concurrently; the inner loop is
software-pipelined (scores[sc+1] before AV[sc]) with kT waves prefetched two
waves ahead. Softmax skips max-subtraction and row sums come free from the
scaled ones-column on V. Out-projections are emitted kc-major (all 8 output
psums live, contraction outer) so the in-order PE queue is not head-of-line
blocked by the last attention wave's normalization. The fp32 residual stream
stays in SBUF for all 4 layers. All biases in the reference are zero and are
folded out.

Self-contained: hardcodes all shapes; no file I/O, no sibling imports.
"""
import numpy as np
import ml_dtypes

import concourse.bass as bass
import concourse.mybir as mybir
import concourse.tile as tile
from concourse import bacc
from concourse import bass_utils

F32 = mybir.dt.float32
F32R = mybir.dt.float32r
BF16 = mybir.dt.bfloat16
F8 = mybir.dt.float8e4
DR = mybir.MatmulPerfMode.DoubleRow
EXP = mybir.ActivationFunctionType.Exp
GELU = mybir.ActivationFunctionType.Gelu

L, D, H, DK, HID = 4, 1024, 16, 64, 4096
B, T, S = 4, 1024, 1024
R = T // 2              # rows (target positions / encoder positions) per core
N_CORES = 8
DC = D // 128           # 8 feature chunks
SC = S // 128           # 8 key chunks
RC = R // 128           # 4 own-row chunks
NP = D // 256           # 4 contraction pairs for DoubleRow
KV_ELEMS = D * R + R * D        # kT flat + v flat per-core shard (bf16)
RG = [[0, 1], [2, 3], [4, 5], [6, 7]]
SW = 32.0               # fp8 weight pre-scale for QKV / cross-KV

_CACHE = {}


def _emit(nc, tc, pools, dram):
    (xp, xbp, x8p, e8p, w8p, w8kp, wbp, qp, kvp, kwp, vap, hp, accp,
     minip, minir, minib) = pools

    def dma(dst, src):
        nc.sync.dma_start(dst, src)

    def dma_s(dst, src):
        # store queue on the Activation HWDGE: keeps collective-input stores
        # out of the weight-prefetch queue (ACT is idle during the KV phases)
        nc.scalar.dma_start(dst, src)

    def r3(t8):
        return t8[:].rearrange("p (c r) -> p c r", r=R)

    def proj8(w8_ap, rhs8, noc, col0, consume, ppool, kcmajor=False):
        """fp8 DoubleRow projection: psum[oc] = sum_p
        W[p-pair, col0+oc*128:+128].T @ rhs8[pair p]; consume(oc, psum).

        w8_ap: [NP, 128, 2*dout] pair-packed fp8 weights.
        rhs8: fp8 tile [128, nc*R] (feature-chunk-major along free dim).
        kcmajor: all `noc` psums live, pair as outer loop (tail-hiding)."""
        rv = r3(rhs8)
        if kcmajor:
            psums = []
            for oc in range(noc):
                ps = ppool.tile([128, R], F32, tag="pj", name=f"p8k{oc}")
                psums.append(ps)
            for p in range(NP):
                ws = w8kp.tile([128, 2 * noc * 128], F8, tag="ws8k")
                wv = ws[:].rearrange("k (ko m) -> k ko m", ko=2)
                c0 = col0
                dma(wv, w8_ap[p].rearrange("k (ko m) -> k ko m", ko=2)
                    [:, :, c0:c0 + noc * 128])
                for oc in range(noc):
                    nc.tensor.matmul(
                        psums[oc][:], wv[:, :, oc * 128:(oc + 1) * 128],
                        rv[:, 2 * p:2 * p + 2, :],
                        start=(p == 0), stop=(p == NP - 1), perf_mode=DR)
            for oc in range(noc):
                consume(oc, psums[oc])
        else:
            for g0 in range(0, noc, 4):
                psums = []
                for _ in range(4):
                    ps = ppool.tile([128, R], F32, tag="pj")
                    psums.append(ps)
                for p in range(NP):
                    ws = w8p.tile([128, 1024], F8, tag="ws8")
                    wv = ws[:].rearrange("k (ko m) -> k ko m", ko=2)
                    c0 = col0 + g0 * 128
                    dma(wv, w8_ap[p].rearrange("k (ko m) -> k ko m", ko=2)
                        [:, :, c0:c0 + 512])
                    for j in range(4):
                        nc.tensor.matmul(
                            psums[j][:], wv[:, :, j * 128:(j + 1) * 128],
                            rv[:, 2 * p:2 * p + 2, :],
                            start=(p == 0), stop=(p == NP - 1), perf_mode=DR)
                for j in range(4):
                    consume(g0 + j, psums[j])

    def attention(q_tiles, kT_of_wave, va_tiles, spool, avpool, escale, avT8):
        """Feature-major attention, software-pipelined over (wave, sc).
        Writes normalized per-wave outputs into avT8 (fp8 [128, 8, R])."""
        NW = H // 2
        kw = [None] * NW
        avs = [None] * NW
        av8 = r3(avT8)
        pend = None  # (w, sc, p_t)

        def ensure_kw(w):
            if w < NW and kw[w] is None:
                kw[w] = kT_of_wave(w)

        def flush(p):
            w_, sc_, pt_ = p
            for hi in range(2):
                nc.tensor.matmul(
                    avs[w_][hi][0:65, :],
                    va_tiles[sc_][:, (2 * w_ + hi) * 65:(2 * w_ + hi + 1) * 65],
                    pt_[:, hi * R:(hi + 1) * R],
                    start=(sc_ == 0), stop=(sc_ == SC - 1))
            if sc_ == SC - 1:
                for hi in range(2):
                    rec = minir.tile([1, R], F32, tag="rec")
                    nc.vector.reciprocal(rec[:], avs[w_][hi][64:65, :])
                    bc = minib.tile([64, R], F32, tag="bc")
                    nc.gpsimd.partition_broadcast(bc[:], rec[:])
                    nc.vector.tensor_mul(
                        av8[hi * 64:(hi + 1) * 64, w_, :],
                        avs[w_][hi][0:64, :], bc[:])

        ensure_kw(0)
        ensure_kw(1)
        for w in range(NW):
            ensure_kw(w + 2)
            avs[w] = [avpool.tile([128, R], F32, tag="av", name=f"av{w}_{i}")
                      for i in range(2)]
            for sc in range(SC):
                slab = spool.tile([128, 2 * R], F32, tag="sc")
                p_t = minip.tile([128, 2 * R], BF16, tag="p")
                for hi in range(2):
                    nc.tensor.matmul(
                        slab[:, hi * R:(hi + 1) * R],
                        kw[w][hi * 64:(hi + 1) * 64, sc * 128:(sc + 1) * 128],
                        q_tiles[w][hi * 64:(hi + 1) * 64, :],
                        start=True, stop=True)
                if pend is not None:
                    flush(pend)
                nc.scalar.activation(p_t[:], slab[:], EXP, scale=escale)
                pend = (w, sc, p_t)
        flush(pend)

    def phase_kv8(li, rhs8, w8_ap, k_col0, v_col0, ccin, pname):
        """fp8 DoubleRow K^T [D, 512] + V [512, D] shard projections, stored
        to ccin (bf16, x32-scaled values) for the pair AllGather."""
        rv = r3(rhs8)
        with tc.tile_pool(name=pname, bufs=8, space="PSUM") as pA:
            def mk_k(oc, ps):
                kt = kvp.tile([128, R], BF16, tag="ko")
                nc.vector.tensor_copy(kt[:], ps[:])
                dma_s(ccin[oc * 128 * R:(oc + 1) * 128 * R]
                      .rearrange("(p s) -> p s", p=128), kt[:])
            proj8(w8_ap, rhs8, DC, k_col0, mk_k, pA)

            for vc in range(2):
                psv = []
                for _ in range(RC):
                    ps = pA.tile([128, 512], F32, tag="pj")
                    psv.append(ps)
                for p in range(NP):
                    ws = w8p.tile([128, 1024], F8, tag="ws8")
                    wv = ws[:].rearrange("k (ko m) -> k ko m", ko=2)
                    dma(wv, w8_ap[p].rearrange("k (ko m) -> k ko m", ko=2)
                        [:, :, v_col0 + vc * 512:v_col0 + (vc + 1) * 512])
                    for t_ in range(RC):
                        nc.tensor.matmul(
                            psv[t_][:],
                            rv[:, 2 * p:2 * p + 2, t_ * 128:(t_ + 1) * 128],
                            wv,
                            start=(p == 0), stop=(p == NP - 1), perf_mode=DR)
                for t_ in range(RC):
                    vt = kvp.tile([128, 512], BF16, tag="vo")
                    nc.vector.tensor_copy(vt[:], psv[t_][:])
                    rbase = D * R + t_ * 128 * D
                    dst = (ccin[rbase:rbase + 128 * D]
                           .rearrange("(p f) -> p f", f=D)
                           [:, vc * 512:(vc + 1) * 512])
                    dma_s(dst, vt[:])

    def phase_q8(li, rhs8, w8_ap, pname):
        qT = [None] * DC
        with tc.tile_pool(name=pname, bufs=8, space="PSUM") as pQ:
            def mk_q(oc, ps):
                t = qp.tile([128, R], BF16, tag="q")
                nc.vector.tensor_copy(t[:], ps[:])
                qT[oc] = t
            proj8(w8_ap, rhs8, DC, 0, mk_q, pQ)
        return qT

    def phase_attn(li, qT, ccout, escale, pname):
        """Attention over a gathered KV buffer: blk0 = pair-even rows,
        blk1 = pair-odd rows. Returns fp8 avT8 [128, 8, R]."""
        va = []
        for sc in range(SC):
            sav = vap.tile([128, H * 65], BF16, tag="sav")
            sav3 = sav[:].rearrange("p (h w) -> p h w", w=65)
            nc.gpsimd.memset(sav3[:, :, 64:65], SW)
            blk = sc // 4
            rbase = blk * KV_ELEMS + D * R + (sc % 4) * 128 * D
            src = (ccout[rbase:rbase + 128 * D]
                   .rearrange("(p f) -> p f", f=D)
                   .rearrange("p (h w) -> p h w", w=DK))
            dma(sav3[:, :, 0:DK], src)
            va.append(sav)

        def kT_wave(w):
            kw = kwp.tile([128, S], BF16, tag="kw")
            for blk in range(2):
                base = blk * KV_ELEMS + w * 128 * R
                dma(kw[:, blk * R:(blk + 1) * R],
                    ccout[base:base + 128 * R]
                    .rearrange("(p s) -> p s", p=128))
            return kw

        avT8 = x8p.tile([128, DC * R], F8, tag="av8", name=f"av8{pname}{li}")
        with (
            tc.tile_pool(name=f"ps{pname}{li}", bufs=2, space="PSUM") as sD,
            tc.tile_pool(name=f"pa{pname}{li}", bufs=4, space="PSUM") as aD,
        ):
            attention(qT, kT_wave, va, sD, aD, escale, avT8)
        return avT8

    def phase_proj_res8(li, name, w8_ap, rhs8, res_tiles, shadows):
        """x_out = W.T @ rhs + res via fp8 DoubleRow, kc-major. `shadows`
        selects extra per-chunk copies: "f8" (fp8 [128,8,R]) or "bf" (bf16
        tiles). Returns (x_tiles, shadow)."""
        xo = [None] * DC
        xb = [None] * DC
        x8 = None
        if shadows == "f8":
            x8 = x8p.tile([128, DC * R], F8, tag="x8", name=f"x8{name}{li}")
            x8v = r3(x8)
        with tc.tile_pool(name=f"ps{name}{li}", bufs=8, space="PSUM") as pp:
            def mk(oc, ps):
                t = xp.tile([128, R], F32R, tag="x")
                nc.vector.tensor_add(t[:], ps[:],
                                     res_tiles[oc][:].bitcast(F32))
                xo[oc] = t
                if shadows == "f8":
                    nc.vector.tensor_copy(x8v[:, oc, :], t[:].bitcast(F32))
                elif shadows == "bf":
                    tb = xbp.tile([128, R], BF16, tag="x2b")
                    nc.vector.tensor_copy(tb[:], t[:].bitcast(F32))
                    xb[oc] = tb
            proj8(w8_ap, rhs8, DC, 0, mk, pp, kcmajor=True)
        return xo, (x8 if shadows == "f8" else xb)

    def phase_ffn(li, wf1, wf2, x2, x2b):
        """bf16 FFN; produces x3 (f32r) + fp8 shadow for the next layer."""
        acc = [None] * DC
        with tc.tile_pool(name=f"psI{li}", bufs=8, space="PSUM") as pI:
            for qtr in range(4):
                hq = [None] * DC
                def mk_h(oc, ps, hq=hq):
                    t = hp.tile([128, R], BF16, tag="h")
                    nc.scalar.activation(t[:], ps[:], GELU)
                    hq[oc] = t
                proj_bf(wf1, x2b, qtr * D, mk_h, pI)
                wf2q = wf2[qtr * D:(qtr + 1) * D, :]
                def mk_acc(oc, ps, qtr=qtr):
                    if qtr == 0:
                        t = accp.tile([128, R], F32, tag="acc")
                        nc.vector.tensor_add(t[:], ps[:],
                                             x2[oc][:].bitcast(F32))
                        acc[oc] = t
                    else:
                        nc.vector.tensor_add(acc[oc][:], ps[:], acc[oc][:])
                proj_bf(wf2q, hq, 0, mk_acc, pI)
        x3 = [None] * DC
        x8n = x8p.tile([128, DC * R], F8, tag="x8", name=f"x8n{li}")
        x8v = r3(x8n)
        for oc in range(DC):
            xt3 = xp.tile([128, R], F32R, tag="x")
            nc.vector.tensor_copy(xt3[:], acc[oc][:])
            x3[oc] = xt3
            nc.vector.tensor_copy(x8v[:, oc, :], acc[oc][:])
        return x3, x8n

    def proj_bf(w_ap, rhs_tiles, col0, consume, ppool):
        """bf16 transposed-mode projection (FFN), groups of 4 out chunks."""
        kcn = len(rhs_tiles)
        for g0 in range(0, DC, 4):
            psums = []
            for _ in range(4):
                ps = ppool.tile([128, R], F32, tag="pj")
                psums.append(ps)
            for kc in range(kcn):
                ws = wbp.tile([128, 512], BF16, tag="wsb")
                c0 = col0 + g0 * 128
                dma(ws[:], w_ap[kc * 128:(kc + 1) * 128, c0:c0 + 512])
                for j in range(4):
                    nc.tensor.matmul(
                        psums[j][:], ws[:, j * 128:(j + 1) * 128],
                        rhs_tiles[kc][:],
                        start=(kc == 0), stop=(kc == kcn - 1))
            for j in range(4):
                consume(g0 + j, psums[j])

    # ---------------- main program ----------------
    (xT_d, xF8_d, encF8_d, w_qkv8, w_o8, w_cq8, w_cakv8, w_co8,
     w_ff1, w_ff2, out_d, cc_in, cc_out, cc2_in, cc2_out) = dram

    xT = []
    for ci in range(DC):
        xt = xp.tile([128, R], F32R, tag="x")
        dma(xt[:], xT_d.ap()[ci * 128:(ci + 1) * 128])
        xT.append(xt)
    xf8 = x8p.tile([128, DC * R], F8, tag="x8", name="x8in")
    xf8v = r3(xf8)
    ef8 = e8p.tile([128, DC * R], F8, tag="e8", name="e8in")
    ef8v = r3(ef8)
    for ci in range(DC):
        dma(xf8v[:, ci, :], xF8_d.ap()[ci * 128:(ci + 1) * 128])
        dma(ef8v[:, ci, :], encF8_d.ap()[ci * 128:(ci + 1) * 128])

    for li in range(L):
        ccin = cc_in[li].ap()
        ccout = cc_out[li].ap()
        ccin2 = cc2_in[li].ap()
        ccout2 = cc2_out[li].ap()
        phase_kv8(li, xf8, w_qkv8.ap()[li], D, 2 * D, ccin, f"psA{li}")
        nc.gpsimd.collective_compute(
            "AllGather", mybir.AluOpType.bypass, replica_groups=RG,
            ins=[ccin], outs=[ccout])
        phase_kv8(li, ef8, w_cakv8.ap()[li], 0, D, ccin2, f"psC{li}")
        qT = phase_q8(li, xf8, w_qkv8.ap()[li], f"psQ{li}")
        avT8 = phase_attn(li, qT, ccout, 0.125 / (SW * SW), "D")
        # AG2 after the SA attention: the collective blocks the GpSimd queue
        # until it completes, so emitting it earlier would stall the SA
        # softmax partition-broadcasts. Output first needed at CA attention.
        nc.gpsimd.collective_compute(
            "AllGather", mybir.AluOpType.bypass, replica_groups=RG,
            ins=[ccin2], outs=[ccout2])
        x1, x1f8 = phase_proj_res8(li, "E", w_o8.ap()[li], avT8, xT, "f8")
        caqT = phase_q8(li, x1f8, w_cq8.ap()[li], f"psF{li}")
        ca_avT8 = phase_attn(li, caqT, ccout2, 0.125 / SW, "G")
        x2, x2b = phase_proj_res8(li, "H", w_co8.ap()[li], ca_avT8, x1, "bf")
        xT, xf8 = phase_ffn(li, w_ff1.ap()[li], w_ff2.ap()[li], x2, x2b)

    for oc in range(DC):
        dma(out_d.ap()[oc * 128:(oc + 1) * 128], xT[oc][:].bitcast(F32))


def _build():
    nc = bacc.Bacc("TRN2", target_bir_lowering=False, debug=False,
                   num_devices=N_CORES)
    dram = (
        nc.dram_tensor("xT", [D, R], F32R, kind="ExternalInput"),
        nc.dram_tensor("xF8", [D, R], F8, kind="ExternalInput"),
        nc.dram_tensor("encF8", [D, R], F8, kind="ExternalInput"),
        nc.dram_tensor("w_qkv8", [L, NP, 128, 2 * 3 * D], F8,
                       kind="ExternalInput"),
        nc.dram_tensor("w_o8", [L, NP, 128, 2 * D], F8, kind="ExternalInput"),
        nc.dram_tensor("w_cq8", [L, NP, 128, 2 * D], F8, kind="ExternalInput"),
        nc.dram_tensor("w_cakv8", [L, NP, 128, 2 * 2 * D], F8,
                       kind="ExternalInput"),
        nc.dram_tensor("w_co8", [L, NP, 128, 2 * D], F8, kind="ExternalInput"),
        nc.dram_tensor("w_ff1", [L, D, HID], BF16, kind="ExternalInput"),
        nc.dram_tensor("w_ff2", [L, HID, D], BF16, kind="ExternalInput"),
        nc.dram_tensor("out", [D, R], F32, kind="ExternalOutput"),
        [nc.dram_tensor(f"cc_in{i}", [KV_ELEMS], BF16, kind="Internal")
         for i in range(L)],
        [nc.dram_tensor(f"cc_out{i}", [2 * KV_ELEMS], BF16, kind="Internal")
         for i in range(L)],
        [nc.dram_tensor(f"cc2_in{i}", [KV_ELEMS], BF16, kind="Internal")
         for i in range(L)],
        [nc.dram_tensor(f"cc2_out{i}", [2 * KV_ELEMS], BF16, kind="Internal")
         for i in range(L)],
    )
    with tile.TileContext(nc) as tc:
        with (
            tc.tile_pool(name="xp", bufs=13) as xp,      # f32r [128,R] resid
            tc.tile_pool(name="xbp", bufs=9) as xbp,     # bf16 [128,R] x2b
            tc.tile_pool(name="x8p", bufs=7) as x8p,     # fp8 [128,8R] shadows
            tc.tile_pool(name="e8p", bufs=1) as e8p,     # fp8 [128,8R] enc
            tc.tile_pool(name="w8p", bufs=8) as w8p,     # fp8 [128,1024] slabs
            tc.tile_pool(name="w8kp", bufs=3) as w8kp,   # fp8 [128,2048] slabs
            tc.tile_pool(name="wbp", bufs=12) as wbp,    # bf16 [128,512] slabs
            tc.tile_pool(name="qp", bufs=8) as qp,       # bf16 [128,R] qT
            tc.tile_pool(name="kvp", bufs=3) as kvp,     # bf16 kv staging
            tc.tile_pool(name="kwp", bufs=3) as kwp,     # bf16 [128,S] kT wave
            tc.tile_pool(name="vap", bufs=8) as vap,     # bf16 [128,H*65] v_aug
            tc.tile_pool(name="hp", bufs=8) as hp,       # bf16 [128,R] ffn hid
            tc.tile_pool(name="accp", bufs=8) as accp,   # f32 [128,R] ffn acc
            tc.tile_pool(name="minip", bufs=4) as minip,  # bf16 p slabs
            tc.tile_pool(name="minir", bufs=2) as minir,  # rec rows
            tc.tile_pool(name="minib", bufs=2) as minib,  # bcast tiles
        ):
            pools = (xp, xbp, x8p, e8p, w8p, w8kp, wbp, qp, kvp, kwp, vap,
                     hp, accp, minip, minir, minib)
            _emit(nc, tc, pools, dram)
    nc.compile()
    return nc


def _get_nc():
    if "nc" not in _CACHE:
        _CACHE["nc"] = _build()
    return _CACHE["nc"]


def _pack8(w, scale):
    """[L, din, dout] float weights -> [L, din//256, 128, 2*dout] e4m3 in
    DoubleRow pair layout: out[l, p, ki, ko*dout+m] = w[l, p*256+ko*128+ki, m]."""
    w = np.asarray(w, np.float32) * scale
    Lw, din, dout = w.shape
    w8 = w.astype(ml_dtypes.float8_e4m3)
    return np.ascontiguousarray(
        w8.reshape(Lw, din // 256, 2, 128, dout).transpose(0, 1, 3, 2, 4)
        .reshape(Lw, din // 256, 128, 2 * dout))


def _prep_in_maps(inputs):
    tgt = np.asarray(inputs["tgt"], dtype=np.float32)
    enc_out = np.asarray(inputs["enc_out"], dtype=np.float32)
    shared = {
        "w_qkv8": _pack8(inputs["sa_qkv_w"], SW),
        "w_o8": _pack8(inputs["sa_out_w"], 1.0),
        "w_cq8": _pack8(inputs["ca_q_w"], 1.0),
        "w_cakv8": _pack8(inputs["ca_kv_w"], SW),
        "w_co8": _pack8(inputs["ca_out_w"], 1.0),
        "w_ff1": np.asarray(inputs["ff_w1"]).astype(ml_dtypes.bfloat16),
        "w_ff2": np.asarray(inputs["ff_w2"]).astype(ml_dtypes.bfloat16),
    }
    in_maps = []
    for c in range(N_CORES):
        b, hh = c // 2, c % 2
        xt = np.ascontiguousarray(tgt[b].T[:, hh * R:(hh + 1) * R])
        et = np.ascontiguousarray(enc_out[b].T[:, hh * R:(hh + 1) * R])
        m = {
            "xT": xt,
            "xF8": xt.astype(ml_dtypes.float8_e4m3),
            "encF8": et.astype(ml_dtypes.float8_e4m3),
        }
        m.update(shared)
        in_maps.append(m)
    return in_maps


def kernel(**inputs):
    nc = _get_nc()
    in_maps = _prep_in_maps(inputs)
    res = bass_utils.run_bass_kernel_spmd(nc, in_maps,
                                          core_ids=list(range(N_CORES)))
    out = np.empty((B, T, D), dtype=np.float32)
    for c in range(N_CORES):
        b, hh = c // 2, c % 2
        out[b, hh * R:(hh + 1) * R, :] = res.results[c]["out"].T
    return out
